# revision 13
# baseline (speedup 1.0000x reference)
"""YOLO-style DetectionLoss on 8 Trainium2 NeuronCores (Bass/Tile).

Pure data parallelism: batch 8192 -> 1024 per core. Per core the
1024*7*7 = 50176 cells are laid out as 128 SBUF partitions x 392 cells
(each partition owns a contiguous run of 8 batch images). All per-cell
math is elementwise along the free dim; work is spread across
DVE/ACT/Pool to balance engine busy time:

  Pool: raw diffs (dxy, dcl), wh sqrt-diff, 4*areas (stt), class premask tail
  DVE : IoU chain (approx reciprocal), responsible-box one-hot via
        reduce-max + is_ge, residual premasks
  ACT : |dx| (scale=2/S), sqrt, and the four Square+accumulate reductions

The per-(term,chunk) partial sums land in a [P, 4*nchunks] accumulator
DMA'd out per core and folded on the host (the scalar "all-reduce").
"""

import numpy as np

import concourse.bacc as bacc
import concourse.mybir as mybir
import concourse.tile as tile
from concourse.bass_utils import run_bass_kernel_spmd

F32 = mybir.dt.float32
AF = mybir.ActivationFunctionType
OP = mybir.AluOpType
AX = mybir.AxisListType

NB, C, S = 3, 20, 7
D = 5 * NB + C                 # 35
B = 8192
NCORES = 8
P = 128

COORD_SCALE, NOOBJ_SCALE = 5.0, 0.5
NTERMS = 4                     # xywh, contain, noobj, class

# class premask channels done on DVE (rest on Pool) — balance knob
CLS_SPLIT = 0


def default_chunks(kpp):
    if kpp % 98 == 0:
        return [98] * (kpp // 98)
    if kpp % 49 == 0:
        return [49] * (kpp // 49)
    if kpp % 7 == 0:
        return [7] * (kpp // 7)
    return [kpp]


def build_nc(bc: int, ks=None, io_bufs: int = 2, loop_repeats: int = 0,
             cls_split: int = CLS_SPLIT, repeats: int = 1):
    """Trace the per-core Bass program for a per-core batch of `bc`."""
    cells = bc * S * S
    assert cells % P == 0
    kpp = cells // P               # cells per partition
    if ks is None:
        ks = default_chunks(kpp)
    assert sum(ks) == kpp
    nchunks = len(ks)

    nc = bacc.Bacc("TRN2", debug=False, num_devices=NCORES)
    out_h = nc.dram_tensor("output", [bc, S, S, D], F32, kind="ExternalInput")
    tgt_h = nc.dram_tensor("target", [bc, S, S, D], F32, kind="ExternalInput")
    acc_h = nc.dram_tensor("acc", [P, NTERMS * nchunks], F32,
                           kind="ExternalOutput")

    out_v = out_h.ap().rearrange("(p a) h w d -> p (a h w d)", p=P)
    tgt_v = tgt_h.ap().rearrange("(p a) h w d -> p (a h w d)", p=P)

    with tile.TileContext(nc) as tc:
        with (
            tc.tile_pool(name="io", bufs=io_bufs) as io_pool,
            tc.tile_pool(name="p6", bufs=2) as p6,
            tc.tile_pool(name="p3", bufs=2) as p3,
            tc.tile_pool(name="p1", bufs=2) as p1,
            tc.tile_pool(name="p12", bufs=2) as p12,
            tc.tile_pool(name="p20", bufs=2) as p20,
            tc.tile_pool(name="accp", bufs=1) as accp,
        ):
            acc = accp.tile([P, NTERMS * nchunks], F32)

            import contextlib
            loop_cm = (tc.For_i(0, loop_repeats, 1) if loop_repeats
                       else contextlib.nullcontext())
            with loop_cm:
              for _rep in range(repeats):
                off = 0
                for ci, k in enumerate(ks):
                    ot = io_pool.tile([P, k * D], F32, name="ot", tag="ot")
                    tt = io_pool.tile([P, k * D], F32, name="tt", tag="tt")
                    nc.sync.dma_start(ot[:], out_v[:, off:off + k * D])
                    nc.sync.dma_start(tt[:], tgt_v[:, off:off + k * D])
                    off += k * D

                    o3 = ot[:].rearrange("p (k d) -> p k d", d=D)
                    t3 = tt[:].rearrange("p (k d) -> p k d", d=D)
                    ob = o3[:, :, 0:15].rearrange("p k (b f) -> p k b f", f=5)
                    tb = t3[:, :, 0:15].rearrange("p k (b f) -> p k b f", f=5)

                    pxy = ob[:, :, :, 0:2]          # [P,k,3,2]
                    pwh = ob[:, :, :, 2:4]
                    pc_ = ob[:, :, :, 4]            # [P,k,3]
                    pcls = o3[:, :, 15:35]          # [P,k,20]
                    txy = tb[:, :, :, 0:2]
                    twh = tb[:, :, :, 2:4]
                    tcls = t3[:, :, 15:35]
                    t0 = tb[:, :, 0, :]             # [P,k,5] target box 0
                    confv = t0[:, :, 4]             # [P,k] exactly 0/1

                    txy0b = t0[:, :, 0:2].unsqueeze(2).broadcast_to([P, k, 3, 2])
                    twh0b = t0[:, :, 2:4].unsqueeze(2).broadcast_to([P, k, 3, 2])
                    cc = p1.tile([P, k], F32, name="cc", tag="cc")[:]
                    nc.vector.tensor_copy(cc, confv)
                    conf = cc
                    conf3 = conf.unsqueeze(2).broadcast_to([P, k, 3])

                    def slot(term):
                        return acc[:, ci * NTERMS + term: ci * NTERMS + term + 1]

                    # ---------- Pool: DMA-dep-only diffs / areas ----------
                    dxy = p6.tile([P, k, 3, 2], F32, name="dxy", tag="dxy")[:]
                    nc.gpsimd.tensor_sub(dxy, pxy, txy)
                    a1 = p3.tile([P, k, 3], F32, name="a1", tag="a1")[:]
                    nc.gpsimd.tensor_mul(a1, ob[:, :, :, 2], ob[:, :, :, 3])
                    a2 = p1.tile([P, k], F32, name="a2", tag="a2")[:]
                    nc.gpsimd.tensor_mul(a2, t0[:, :, 2], t0[:, :, 3])
                    nc.gpsimd.tensor_add(                # s12, in place
                        a1, a1, a2.unsqueeze(2).broadcast_to([P, k, 3]))
                    dcl = p20.tile([P, k, 20], F32, name="dcl", tag="dcl")[:]
                    nc.vector.tensor_sub(dcl, pcls, tcls)

                    # ---------- DVE: IoU chain ----------
                    dcx = p6.tile([P, k, 3, 2], F32, name="dcx", tag="dcx")[:]
                    nc.vector.tensor_sub(dcx, pxy, txy0b)
                    s6 = p6.tile([P, k, 3, 2], F32, name="s6", tag="s6")[:]
                    mn = p6.tile([P, k, 3, 2], F32, name="mn", tag="mn")[:]
                    nc.vector.tensor_add(s6, pwh, twh0b)
                    nc.vector.tensor_tensor(mn, pwh, twh0b, op=OP.min)

                    # ---------- ACT: early unaries ----------
                    nc.scalar.activation(dcx, dcx, AF.Abs, scale=2.0 / S)
                    sp = p6.tile([P, k, 3, 2], F32, name="sp", tag="sp")[:]
                    nc.scalar.activation(sp, pwh, AF.Sqrt)
                    st = p6.tile([P, k, 3, 2], F32, name="st", tag="st")[:]
                    nc.scalar.activation(st, twh, AF.Sqrt)

                    # DVE: ov2 = relu(min(2*min(pw,tw), pw+tw-|2dx/S|))
                    nc.vector.tensor_sub(s6, s6, dcx)         # u, in place
                    nc.vector.scalar_tensor_tensor(           # ov, in place
                        mn, mn, 2.0, s6, op0=OP.mult, op1=OP.min)
                    mnf = mn.rearrange("p k b f -> p (k b f)")
                    nc.vector.tensor_scalar_max(mnf, mnf, 0.0)  # relu, 2x mode
                    inter = p3.tile([P, k, 3], F32, name="inter", tag="inter")[:]
                    nc.vector.tensor_mul(inter, mn[:, :, :, 0], mn[:, :, :, 1])
                    nc.vector.scalar_tensor_tensor(           # den4, in place
                        a1, a1, 4.0, inter, op0=OP.mult, op1=OP.subtract)
                    rcp = p3.tile([P, k, 3], F32, name="rcp", tag="rcp")[:]
                    nc.vector.reciprocal_approx_fast(
                        rcp.rearrange("p k b -> p (k b)"),
                        a1.rearrange("p k b -> p (k b)"))
                    iou = inter                               # in place
                    nc.vector.tensor_mul(iou, inter, rcp)

                    # ---------- responsible-box one-hot (per-box [P,k]
                    # compares; innermost strides stay small) ----------
                    mx = p1.tile([P, k], F32, name="mx", tag="mx")[:]
                    nc.vector.tensor_tensor(mx, iou[:, :, 0], iou[:, :, 1],
                                            op=OP.max)
                    nc.vector.tensor_tensor(mx, mx, iou[:, :, 2], op=OP.max)
                    rm = p3.tile([P, k, 3], F32, name="rm", tag="rm")[:]
                    for b in range(3):
                        nc.vector.tensor_tensor(rm[:, :, b], iou[:, :, b], mx,
                                                op=OP.is_ge)
                    for b in range(3):
                        nc.vector.tensor_mul(rm[:, :, b], rm[:, :, b], conf)
                    rm2 = rm.unsqueeze(3).broadcast_to([P, k, 3, 2])

                    # ---------- Pool: wh sqrt diff (needs ACT sp/st) ----
                    nc.gpsimd.tensor_sub(sp, sp, st)   # dwh, in place

                    # ---------- masked residuals ----------
                    cw = p12.tile([P, k, 3, 4], F32, name="cw", tag="cw")[:]
                    nc.vector.tensor_mul(cw[:, :, :, 0:2], dxy, rm2)
                    nc.vector.tensor_mul(cw[:, :, :, 2:4], sp, rm2)
                    nc.vector.tensor_sub(iou, pc_, iou)       # dc, in place
                    nc.vector.tensor_mul(iou, iou, rm)        # dcm, in place
                    pcm = p3.tile([P, k, 3], F32, name="pcm", tag="pcm")[:]
                    for b in range(3):
                        nc.vector.scalar_tensor_tensor(
                            pcm[:, :, b], conf, 1.0, pc_[:, :, b],
                            op0=OP.not_equal, op1=OP.mult)
                    mdcl = p20.tile([P, k, 20], F32, name="mdcl", tag="mdcl")[:]
                    conf20 = conf.unsqueeze(2).broadcast_to([P, k, 20])
                    cs = cls_split
                    if cs > 0:
                        nc.vector.tensor_mul(
                            mdcl[:, :, 0:cs], dcl[:, :, 0:cs],
                            conf20[:, :, 0:cs])
                    if cs < 20:
                        nc.gpsimd.tensor_mul(
                            mdcl[:, :, cs:20], dcl[:, :, cs:20],
                            conf20[:, :, cs:20])

                    # ---------- ACT: chunk-closing Square+accumulate ----
                    nc.scalar.activation(cw, cw, AF.Square, accum_out=slot(0))
                    nc.scalar.activation(iou, iou, AF.Square, accum_out=slot(1))
                    nc.scalar.activation(pcm, pcm, AF.Square, accum_out=slot(2))
                    nc.scalar.activation(mdcl, mdcl, AF.Square,
                                         accum_out=slot(3))

            nc.sync.dma_start(acc_h.ap()[:], acc[:])

    nc.compile()
    return nc


_CACHE = {}


def _get_nc(bc, ks=None, io_bufs=2, loop_repeats=0, cls_split=CLS_SPLIT,
            repeats=1, **_ignored):
    key = (bc, tuple(ks) if ks else None, io_bufs, loop_repeats, cls_split,
           repeats)
    if key not in _CACHE:
        _CACHE[key] = build_nc(bc, ks, io_bufs, loop_repeats, cls_split,
                               repeats)
    return _CACHE[key]


def combine_acc(acc_list, nchunks):
    """Host-side gather: fold per-(core,partition,chunk) term sums into the
    scalar loss exactly as the reference's final weighted sum does."""
    tot = np.zeros(NTERMS, dtype=np.float64)
    for a in acc_list:
        tot += a.astype(np.float64).reshape(P, nchunks, NTERMS).sum(axis=(0, 1))
    xywh, cont, noobj, cls = tot
    loss = (COORD_SCALE * xywh + cont + NOOBJ_SCALE * noobj + cls) / B
    return np.float32(loss)


BEST_KS = [49, 98, 98, 98, 49]
BEST_IO_BUFS = 3
def extra_inputs():
    return {}


def kernel(output: np.ndarray, target: np.ndarray) -> np.ndarray:
    assert output.shape == (B, S, S, D) and target.shape == (B, S, S, D)
    bc = B // NCORES
    nchunks = len(BEST_KS)
    nc = _get_nc(bc, BEST_KS, io_bufs=BEST_IO_BUFS)
    in_maps = [
        {
            "output": np.ascontiguousarray(output[i * bc:(i + 1) * bc]),
            "target": np.ascontiguousarray(target[i * bc:(i + 1) * bc]),
        }
        for i in range(NCORES)
    ]
    res = run_bass_kernel_spmd(nc, in_maps, list(range(NCORES)))
    return combine_acc([r["acc"] for r in res.results], nchunks)


# revision 15
# speedup vs baseline: 1.1094x; 1.1094x over previous
"""YOLO-style DetectionLoss on 8 Trainium2 NeuronCores (Bass/Tile).

Pure data parallelism: batch 8192 -> 1024 per core. Per core the
1024*7*7 = 50176 cells are laid out as 128 SBUF partitions x 392 cells
(each partition owns a contiguous run of 8 batch images). All per-cell
math is elementwise along the free dim; work is spread across
DVE/ACT/Pool to balance engine busy time:

  Pool: raw diffs (dxy, dcl), wh sqrt-diff, 4*areas (stt), class premask tail
  DVE : IoU chain (approx reciprocal), responsible-box one-hot via
        reduce-max + is_ge, residual premasks
  ACT : |dx| (scale=2/S), sqrt, and the four Square+accumulate reductions

The per-(term,chunk) partial sums land in a [P, 4*nchunks] accumulator
DMA'd out per core and folded on the host (the scalar "all-reduce").
"""

import numpy as np

import concourse.bacc as bacc
import concourse.mybir as mybir
import concourse.tile as tile
from concourse.bass_utils import run_bass_kernel_spmd

F32 = mybir.dt.float32
AF = mybir.ActivationFunctionType
OP = mybir.AluOpType
AX = mybir.AxisListType

NB, C, S = 3, 20, 7
D = 5 * NB + C                 # 35
B = 8192
NCORES = 8
P = 128

COORD_SCALE, NOOBJ_SCALE = 5.0, 0.5
NTERMS = 4                     # xywh, contain, noobj, class

# class premask channels done on DVE (rest on Pool) — balance knob
CLS_SPLIT = 0


def default_chunks(kpp):
    if kpp % 98 == 0:
        return [98] * (kpp // 98)
    if kpp % 49 == 0:
        return [49] * (kpp // 49)
    if kpp % 7 == 0:
        return [7] * (kpp // 7)
    return [kpp]


def build_nc(bc: int, ks=None, io_bufs: int = 2, loop_repeats: int = 0,
             cls_split: int = CLS_SPLIT, repeats: int = 1):
    """Trace the per-core Bass program for a per-core batch of `bc`."""
    cells = bc * S * S
    assert cells % P == 0
    kpp = cells // P               # cells per partition
    if ks is None:
        ks = default_chunks(kpp)
    assert sum(ks) == kpp
    nchunks = len(ks)

    nc = bacc.Bacc("TRN2", debug=False, num_devices=NCORES)
    out_h = nc.dram_tensor("output", [bc, S, S, D], F32, kind="ExternalInput")
    tgt_h = nc.dram_tensor("target", [bc, S, S, D], F32, kind="ExternalInput")
    acc_h = nc.dram_tensor("acc", [P, NTERMS * nchunks], F32,
                           kind="ExternalOutput")

    out_v = out_h.ap().rearrange("(p a) h w d -> p (a h w d)", p=P)
    tgt_v = tgt_h.ap().rearrange("(p a) h w d -> p (a h w d)", p=P)

    with tile.TileContext(nc) as tc:
        with (
            tc.tile_pool(name="io", bufs=io_bufs) as io_pool,
            tc.tile_pool(name="p6", bufs=2) as p6,
            tc.tile_pool(name="p3", bufs=2) as p3,
            tc.tile_pool(name="p1", bufs=2) as p1,
            tc.tile_pool(name="p12", bufs=2) as p12,
            tc.tile_pool(name="p20", bufs=2) as p20,
            tc.tile_pool(name="accp", bufs=1) as accp,
        ):
            acc = accp.tile([P, NTERMS * nchunks], F32)

            import contextlib
            loop_cm = (tc.For_i(0, loop_repeats, 1) if loop_repeats
                       else contextlib.nullcontext())
            with loop_cm:
              for _rep in range(repeats):
                off = 0
                for ci, k in enumerate(ks):
                    ot = io_pool.tile([P, k * D], F32, name="ot", tag="ot")
                    tt = io_pool.tile([P, k * D], F32, name="tt", tag="tt")
                    nc.sync.dma_start(ot[:], out_v[:, off:off + k * D])
                    nc.sync.dma_start(tt[:], tgt_v[:, off:off + k * D])
                    off += k * D

                    o3 = ot[:].rearrange("p (k d) -> p k d", d=D)
                    t3 = tt[:].rearrange("p (k d) -> p k d", d=D)
                    ob = o3[:, :, 0:15].rearrange("p k (b f) -> p k b f", f=5)
                    tb = t3[:, :, 0:15].rearrange("p k (b f) -> p k b f", f=5)

                    pxy = ob[:, :, :, 0:2]          # [P,k,3,2]
                    pwh = ob[:, :, :, 2:4]
                    pc_ = ob[:, :, :, 4]            # [P,k,3]
                    pcls = o3[:, :, 15:35]          # [P,k,20]
                    txy = tb[:, :, :, 0:2]
                    twh = tb[:, :, :, 2:4]
                    tcls = t3[:, :, 15:35]
                    t0 = tb[:, :, 0, :]             # [P,k,5] target box 0
                    confv = t0[:, :, 4]             # [P,k] exactly 0/1

                    txy0b = t0[:, :, 0:2].unsqueeze(2).broadcast_to([P, k, 3, 2])
                    twh0b = t0[:, :, 2:4].unsqueeze(2).broadcast_to([P, k, 3, 2])
                    cc = p1.tile([P, k], F32, name="cc", tag="cc")[:]
                    nc.vector.tensor_copy(cc, confv)
                    conf = cc
                    conf3 = conf.unsqueeze(2).broadcast_to([P, k, 3])

                    def slot(term):
                        return acc[:, ci * NTERMS + term: ci * NTERMS + term + 1]

                    # ---------- Pool: DMA-dep-only diffs / areas ----------
                    dxy = p6.tile([P, k, 3, 2], F32, name="dxy", tag="dxy")[:]
                    nc.gpsimd.tensor_sub(dxy, pxy, txy)
                    a1 = p3.tile([P, k, 3], F32, name="a1", tag="a1")[:]
                    nc.gpsimd.tensor_mul(a1, ob[:, :, :, 2], ob[:, :, :, 3])
                    a2 = p1.tile([P, k], F32, name="a2", tag="a2")[:]
                    nc.gpsimd.tensor_mul(a2, t0[:, :, 2], t0[:, :, 3])
                    nc.gpsimd.tensor_add(                # s12, in place
                        a1, a1, a2.unsqueeze(2).broadcast_to([P, k, 3]))
                    dcl = p20.tile([P, k, 20], F32, name="dcl", tag="dcl")[:]
                    nc.vector.tensor_sub(dcl, pcls, tcls)

                    # ---------- DVE: IoU chain ----------
                    dcx = p6.tile([P, k, 3, 2], F32, name="dcx", tag="dcx")[:]
                    nc.vector.tensor_sub(dcx, pxy, txy0b)
                    s6 = p6.tile([P, k, 3, 2], F32, name="s6", tag="s6")[:]
                    mn = p6.tile([P, k, 3, 2], F32, name="mn", tag="mn")[:]
                    nc.vector.tensor_add(s6, pwh, twh0b)
                    nc.vector.tensor_tensor(mn, pwh, twh0b, op=OP.min)

                    # ---------- ACT: early unaries ----------
                    nc.scalar.activation(dcx, dcx, AF.Abs, scale=2.0 / S)
                    sp = p6.tile([P, k, 3, 2], F32, name="sp", tag="sp")[:]
                    nc.scalar.activation(sp, pwh, AF.Sqrt)
                    st = p6.tile([P, k, 3, 2], F32, name="st", tag="st")[:]
                    nc.scalar.activation(st, twh, AF.Sqrt)

                    # DVE: ov2 = relu(min(2*min(pw,tw), pw+tw-|2dx/S|))
                    u6 = p6.tile([P, k, 3, 2], F32, name="u6", tag="u6")[:]
                    nc.vector.tensor_sub(u6, s6, dcx)
                    mn2 = p6.tile([P, k, 3, 2], F32, name="mn2", tag="mn2")[:]
                    nc.vector.tensor_scalar_mul(
                        mn2.rearrange("p k b f -> p (k b f)"),
                        mn.rearrange("p k b f -> p (k b f)"), 2.0)  # 2x mode
                    ov = p6.tile([P, k, 3, 2], F32, name="ov", tag="ov")[:]
                    nc.vector.tensor_tensor(ov, mn2, u6, op=OP.min)
                    ovf = ov.rearrange("p k b f -> p (k b f)")
                    nc.vector.tensor_scalar_max(ovf, ovf, 0.0)  # relu, 2x mode
                    inter = p3.tile([P, k, 3], F32, name="inter", tag="inter")[:]
                    nc.vector.tensor_mul(inter, ov[:, :, :, 0], ov[:, :, :, 1])
                    den = p3.tile([P, k, 3], F32, name="den", tag="den")[:]
                    nc.vector.scalar_tensor_tensor(
                        den, a1, 4.0, inter, op0=OP.mult, op1=OP.subtract)
                    rcp = p3.tile([P, k, 3], F32, name="rcp", tag="rcp")[:]
                    nc.vector.reciprocal_approx_fast(
                        rcp.rearrange("p k b -> p (k b)"),
                        den.rearrange("p k b -> p (k b)"))
                    iou = p3.tile([P, k, 3], F32, name="iou", tag="iou")[:]
                    nc.vector.tensor_mul(iou, inter, rcp)

                    # ---------- responsible-box one-hot (per-box [P,k]
                    # compares; innermost strides stay small) ----------
                    mx = p1.tile([P, k], F32, name="mx", tag="mx")[:]
                    nc.vector.tensor_tensor(mx, iou[:, :, 0], iou[:, :, 1],
                                            op=OP.max)
                    nc.vector.tensor_tensor(mx, mx, iou[:, :, 2], op=OP.max)
                    rm = p3.tile([P, k, 3], F32, name="rm", tag="rm")[:]
                    for b in range(3):
                        nc.vector.tensor_tensor(rm[:, :, b], iou[:, :, b], mx,
                                                op=OP.is_ge)
                    for b in range(3):
                        nc.vector.tensor_mul(rm[:, :, b], rm[:, :, b], conf)

                    # ---------- Pool: wh sqrt diff (needs ACT sp/st) ----
                    nc.gpsimd.tensor_sub(sp, sp, st)   # dwh, in place

                    # ---------- masked residuals ----------
                    cw = p12.tile([P, k, 4, 3], F32, name="cw", tag="cw")[:]
                    rm2m = rm.unsqueeze(2).broadcast_to([P, k, 2, 3])
                    nc.vector.tensor_mul(
                        cw[:, :, 0:2, :],
                        dxy.rearrange("p k b f -> p k f b"), rm2m)
                    nc.vector.tensor_mul(
                        cw[:, :, 2:4, :],
                        sp.rearrange("p k b f -> p k f b"), rm2m)
                    dct = p3.tile([P, k, 3], F32, name="dct", tag="dct")[:]
                    nc.vector.tensor_sub(dct, pc_, iou)
                    dcm = p3.tile([P, k, 3], F32, name="dcm", tag="dcm")[:]
                    nc.vector.tensor_mul(dcm, dct, rm)
                    pcm = p3.tile([P, k, 3], F32, name="pcm", tag="pcm")[:]
                    for b in range(3):
                        nc.vector.scalar_tensor_tensor(
                            pcm[:, :, b], conf, 1.0, pc_[:, :, b],
                            op0=OP.not_equal, op1=OP.mult)
                    mdcl = p20.tile([P, k, 20], F32, name="mdcl", tag="mdcl")[:]
                    conf20 = conf.unsqueeze(2).broadcast_to([P, k, 20])
                    cs = cls_split
                    if cs > 0:
                        nc.vector.tensor_mul(
                            mdcl[:, :, 0:cs], dcl[:, :, 0:cs],
                            conf20[:, :, 0:cs])
                    if cs < 20:
                        nc.gpsimd.tensor_mul(
                            mdcl[:, :, cs:20], dcl[:, :, cs:20],
                            conf20[:, :, cs:20])

                    # ---------- ACT: chunk-closing Square+accumulate ----
                    nc.scalar.activation(cw, cw, AF.Square, accum_out=slot(0))
                    nc.scalar.activation(dcm, dcm, AF.Square, accum_out=slot(1))
                    nc.scalar.activation(pcm, pcm, AF.Square, accum_out=slot(2))
                    nc.scalar.activation(mdcl, mdcl, AF.Square,
                                         accum_out=slot(3))

            nc.sync.dma_start(acc_h.ap()[:], acc[:])

    nc.compile()
    return nc


_CACHE = {}


def _get_nc(bc, ks=None, io_bufs=2, loop_repeats=0, cls_split=CLS_SPLIT,
            repeats=1, **_ignored):
    key = (bc, tuple(ks) if ks else None, io_bufs, loop_repeats, cls_split,
           repeats)
    if key not in _CACHE:
        _CACHE[key] = build_nc(bc, ks, io_bufs, loop_repeats, cls_split,
                               repeats)
    return _CACHE[key]


def combine_acc(acc_list, nchunks):
    """Host-side gather: fold per-(core,partition,chunk) term sums into the
    scalar loss exactly as the reference's final weighted sum does."""
    tot = np.zeros(NTERMS, dtype=np.float64)
    for a in acc_list:
        tot += a.astype(np.float64).reshape(P, nchunks, NTERMS).sum(axis=(0, 1))
    xywh, cont, noobj, cls = tot
    loss = (COORD_SCALE * xywh + cont + NOOBJ_SCALE * noobj + cls) / B
    return np.float32(loss)


BEST_KS = [49, 98, 98, 98, 49]
BEST_IO_BUFS = 3
def extra_inputs():
    return {}


def kernel(output: np.ndarray, target: np.ndarray) -> np.ndarray:
    assert output.shape == (B, S, S, D) and target.shape == (B, S, S, D)
    bc = B // NCORES
    nchunks = len(BEST_KS)
    nc = _get_nc(bc, BEST_KS, io_bufs=BEST_IO_BUFS)
    in_maps = [
        {
            "output": np.ascontiguousarray(output[i * bc:(i + 1) * bc]),
            "target": np.ascontiguousarray(target[i * bc:(i + 1) * bc]),
        }
        for i in range(NCORES)
    ]
    res = run_bass_kernel_spmd(nc, in_maps, list(range(NCORES)))
    return combine_acc([r["acc"] for r in res.results], nchunks)
